# revision 1
# baseline (speedup 1.0000x reference)
"""Trainium2 Bass kernel for nn_Mnist_lmdSplineKAN.

Sharding: data-parallel over batch, 8 cores x 128 rows. All params replicated.

Per-core math (I=784 inputs, H=10 heads, O=64, 8 B-spline basis fns, order 3,
5 uniform intervals on [0,1)):
  t = floor(5x) (int-round trick), u = 5x - t, one-hot masks m_t = (t == const)
  features[b,i,j] = sum_t m_t * p_{j-t}(u)  with p = 6x local cubic polys
  features[b,i,8] = silu(x[b,i])
  y[b,(h,o)] = sum_{i,j} features[b,i,j] * Wbig[(i,j),(h,o)]  (fp16 matmul;
               Wbig folds coef*scale_sp*lmd/6 and scale_base*lmd)
  h1 = tanh(y); h2 = tanh(h1 @ blockdiag(W1) + b1); logits = <h2,W2>_head + b2

I is tiled as 6 chunks of 128 (full partitions, FWL-eligible) + 1 of 16.
Weights stream as per-chunk piece-major contiguous DMAs on the SWDGE queue;
matmuls are emitted in a wavefront order matching weight-arrival (c) and
feature-completion (j) times so the PE FIFO never head-blocks.
"""
import sys, types
import numpy as np

B, I, O, H, NB = 1024, 784, 64, 10, 8
NC = 8
BC = B // NC      # 128
CH = 7            # 6 full 128-row chunks + 1 of 16
PLAST = 16
HO = H * O        # 640
D2 = H * 32       # 320
NH = 2


def _install_ntff_hook():
    if "antenv.axon_hooks" in sys.modules:
        return
    try:
        import antenv
        mod = types.ModuleType("antenv.axon_hooks")
        _h = [None]
        mod.set_axon_ntff_profile_hook = lambda h: _h.__setitem__(0, h)
        mod.get_axon_ntff_profile_hook = lambda: _h[0]
        sys.modules["antenv.axon_hooks"] = mod
        antenv.axon_hooks = mod
        from trn_agent_boot.trn_boot import _ntff_profile_via_ctypes
        h = _ntff_profile_via_ctypes("/opt/axon/libaxon_pjrt.so")
        if h is not None:
            mod.set_axon_ntff_profile_hook(h)
    except Exception:
        pass


_CACHE = {}


def _build():
    if "nc" in _CACHE:
        return _CACHE["nc"]
    import concourse.bacc as bacc
    import concourse.bass as bass
    import concourse.tile as tile
    from concourse import mybir
    from contextlib import ExitStack

    f32, f16, i32 = mybir.dt.float32, mybir.dt.float16, mybir.dt.int32
    ALU = mybir.AluOpType
    AF = mybir.ActivationFunctionType

    nc = bacc.Bacc("TRN2", target_bir_lowering=False, debug=False)
    x_d = nc.dram_tensor("x", (128, CH, BC), f32, kind="ExternalInput").ap()
    w_d = nc.dram_tensor("w", (I * (NB + 1) * HO,), f16,
                         kind="ExternalInput").ap()
    w1_d = nc.dram_tensor("w1", (128, 5 * D2 + 128), f16,
                          kind="ExternalInput").ap()
    b1_d = nc.dram_tensor("b1", (1, D2), f16, kind="ExternalInput").ap()
    w2_d = nc.dram_tensor("w2", (128, D2 + H), f32, kind="ExternalInput").ap()
    out_d = nc.dram_tensor("out", (BC, H), f32, kind="ExternalOutput").ap()

    with tile.TileContext(nc) as tc, ExitStack() as ctx:
        sb = ctx.enter_context(tc.tile_pool(name="sb", bufs=1))
        ps = ctx.enter_context(tc.tile_pool(name="ps", bufs=1, space="PSUM"))

        # ---- x split across both HWDGE queues: lands first ----
        xt = sb.tile([128, CH, BC], f32, tag="xt")
        nc.sync.dma_start(xt[:, 0:4, :], x_d[:, 0:4, :])
        nc.scalar.dma_start(xt[:, 4:CH, :], x_d[:, 4:CH, :])
        ones = sb.tile([1, 128], f16, tag="ones")
        nc.vector.memset(ones[:], 1.0)

        # ---- weights: piece-major contiguous pieces on the SWDGE queue in
        #      consumption order; last (16-row) chunk split by output half ----
        ROW = (NB + 1) * HO
        wg = []
        off = 0
        for c in range(6):
            t = sb.tile([128, NB + 1, HO], f16, tag=f"wg{c}", name=f"wg{c}")
            src = bass.AP(tensor=w_d.tensor, offset=off,
                          ap=[[ROW, 128], [1, ROW]])
            nc.gpsimd.dma_start(t[:], src)
            wg.append(t)
            off += 128 * ROW
        wg6 = []
        for nh in range(NH):
            t = sb.tile([PLAST, NB + 1, D2], f16, tag=f"wg6{nh}",
                        name=f"wg6{nh}")
            run = (NB + 1) * D2
            src = bass.AP(tensor=w_d.tensor, offset=off,
                          ap=[[run, PLAST], [1, run]])
            nc.gpsimd.dma_start(t[:], src)
            wg6.append(t)
            off += PLAST * run

        def wslice(c, j, nh):
            if c < 6:
                return wg[c][:, j, nh * D2:(nh + 1) * D2]
            return wg6[nh][:, j, :]

        # ---- tail consts, trailing on the SWDGE queue ----
        c16 = sb.tile([128, 5 * D2 + 128], f16, tag="c16")
        nc.gpsimd.dma_start(c16[:], w1_d)
        w1t = c16[:, 0:5 * D2].rearrange("p (k d) -> p k d", d=D2)
        idt = c16[:, 5 * D2:]
        c32 = sb.tile([128, D2 + H], f32, tag="c32")
        nc.gpsimd.dma_start(c32[:], w2_d)
        w2b = c32[:, 0:D2]
        b2b = c32[:, D2:]
        b1r = sb.tile([1, D2], f16, tag="b1r")
        nc.gpsimd.dma_start(b1r[:], b1_d)

        x = xt[:].rearrange("p c b -> p (c b)")

        def T(tag, dt=f16):
            return sb.tile([128, CH * BC], dt, tag=tag, name=tag)

        # ---- features tiles; silu first on ACT (only needs x) ----
        f_ = {}
        for j in range(NB):
            f_[j] = sb.tile([128, CH, BC], f16, tag=f"f{j}", name=f"f{j}")
        fs = sb.tile([128, CH, BC], f16, tag="f8")
        nc.scalar.activation(fs[:].rearrange("p c b -> p (c b)"), x, AF.Silu)
        f_[NB] = fs

        # ---- interval index t = floor(5x) via round(5x-0.5); u; masks ----
        ti = T("ti", i32)
        nc.vector.tensor_scalar(ti[:], x, 5.0, -0.5, op0=ALU.mult, op1=ALU.add)
        u = T("u", f32)
        nc.vector.scalar_tensor_tensor(u[:], x, 5.0, ti[:],
                                       op0=ALU.mult, op1=ALU.subtract)
        M = sb.tile([128, 5, CH * BC], f16, tag="M")
        for t in range(5):
            nc.vector.tensor_scalar(M[:, t, :], ti[:], t, None, op0=ALU.is_equal)

        # ---- local cubics (x6): p0=(1-u)^3, p1=(3u-6)u^2+4, p2=p1(1-u),
        #      p3=u^3;  ACT makes f16 operands, DVE multiplies at 2x ----
        u_h = T("u_h"); nc.scalar.activation(u_h[:], u[:], AF.Copy)
        u2h = T("u2h"); nc.scalar.activation(u2h[:], u[:], AF.Square)
        w_ = T("w_")
        nc.scalar.activation(w_[:], u[:], AF.Copy, bias=1.0, scale=-1.0)
        w2h = T("w2h"); nc.scalar.activation(w2h[:], w_[:], AF.Square)
        a_ = T("a_")
        nc.scalar.activation(a_[:], u[:], AF.Copy, bias=-6.0, scale=3.0)
        b_ = T("b_")
        nc.scalar.activation(b_[:], w_[:], AF.Copy, bias=-6.0, scale=3.0)
        PR = sb.tile([128, 4, CH * BC], f16, tag="PR")
        nc.vector.tensor_tensor(PR[:, 0, :], u2h[:], u_h[:], op=ALU.mult)   # p3
        nc.vector.tensor_tensor(PR[:, 3, :], w2h[:], w_[:], op=ALU.mult)    # p0
        p1pre = T("p1pre")
        nc.vector.tensor_tensor(p1pre[:], a_[:], u2h[:], op=ALU.mult)
        nc.scalar.activation(PR[:, 2, :], p1pre[:], AF.Copy, bias=4.0, scale=1.0)
        p2pre = T("p2pre")
        nc.vector.tensor_tensor(p2pre[:], b_[:], w2h[:], op=ALU.mult)
        nc.scalar.activation(PR[:, 1, :], p2pre[:], AF.Copy, bias=4.0, scale=1.0)

        psum = [ps.tile([128, D2], f32, tag=f"y{nh}", name=f"y{nh}")
                for nh in range(NH)]

        JORDER = (0, 7, 1, 6, 2, 5, 3, 4)
        tk = sb.tile([128, 4, CH * BC], f16, tag="tk")
        t2 = sb.tile([128, 2, CH * BC], f16, tag="t2")
        tmp = T("tmp")
        for j in JORDER:
            tlo = max(0, j - 3)
            k = min(4, j) - tlo + 1
            s0 = 3 - min(j, 3)
            out = f_[j][:].rearrange("p c b -> p (c b)")
            if k == 1:
                nc.vector.tensor_tensor(out, M[:, tlo, :], PR[:, s0, :],
                                        op=ALU.mult)
                continue
            nc.vector.tensor_tensor(tk[:, 0:k, :], M[:, tlo:tlo + k, :],
                                    PR[:, s0:s0 + k, :], op=ALU.mult)
            if k == 2:
                nc.vector.tensor_tensor(out, tk[:, 0, :], tk[:, 1, :], op=ALU.add)
            elif k == 3:
                nc.vector.tensor_tensor(tmp[:], tk[:, 0, :], tk[:, 1, :], op=ALU.add)
                nc.vector.tensor_tensor(out, tmp[:], tk[:, 2, :], op=ALU.add)
            else:
                nc.vector.tensor_tensor(t2[:], tk[:, 0:2, :], tk[:, 2:4, :],
                                        op=ALU.add)
                nc.vector.tensor_tensor(out, t2[:, 0, :], t2[:, 1, :], op=ALU.add)

        # ---- main matmuls in wavefront order ----
        # feature completion follows JORDER; cumulative DVE ops to finish j
        cumops = {}
        acc = 0
        for j in JORDER:
            acc += 2 * len([t for t in range(5) if 0 <= j - t <= 3]) - 1
            cumops[j] = acc

        cumm = {}
        acc = 0
        for j in JORDER:
            k = len([t for t in range(5) if 0 <= j - t <= 3])
            acc += k
            cumm[j] = (acc, k)

        RJ = {0: 4.0, 7: 4.6, 1: 6.5, 6: 8.2, 2: 10.8, 5: 13.5,
              3: 17.2, 4: 20.7, NB: -3.0}

        def ready(cj):
            c, j = cj
            return max(3.45 * c, RJ[j])
        order = sorted(((c, j) for c in range(CH) for j in range(NB + 1)),
                       key=lambda cj: (ready(cj), cj[0]))
        NTOT = CH * (NB + 1)
        for nmm, (c, j) in enumerate(order):
            lhs = f_[j][:, c, :] if c < 6 else f_[j][0:PLAST, c, :]
            for nh in range(NH):
                nc.tensor.matmul(
                    psum[nh][:], lhs, wslice(c, j, nh),
                    start=(nmm == 0), stop=(nmm == NTOT - 1))

        # ---- tail: h1 = tanh(y), transpose, blockdiag MLP, reduce ----
        h1 = sb.tile([128, HO], f16, tag="h1")
        SEG = [(0, 0, 128), (0, 128, 256), (0, 256, 320), (1, 320, 384),
               (1, 384, 512), (1, 512, 640)]

        def tanh_seg(k):
            nh, s0, s1 = SEG[k]
            nc.scalar.activation(h1[:, s0:s1],
                                 psum[nh][:, s0 - nh * D2:s1 - nh * D2],
                                 AF.Tanh)

        h1t = []

        def tr(k):
            pt = ps.tile([128, 128], f16, tag=f"pt{k}", name=f"pt{k}")
            nc.tensor.transpose(pt[:], h1[:, k * 128:(k + 1) * 128], idt)
            st = sb.tile([128, 128], f16, tag=f"h1t{k}", name=f"h1t{k}")
            nc.vector.tensor_copy(st[:], pt[:])
            h1t.append(st)

        tanh_seg(0); tr(0)
        tanh_seg(1); tr(1)
        tanh_seg(2); tanh_seg(3); tr(2)
        tanh_seg(4); tr(3)
        tanh_seg(5); tr(4)

        ps2 = ps.tile([128, D2], f32, tag="ps2")
        for k in range(5):
            nc.tensor.matmul(ps2[:], h1t[k][:], w1t[:, k, :],
                             start=(k == 0), stop=False)
        nc.tensor.matmul(ps2[:], ones[:], b1r[:], start=False, stop=True)
        h2 = sb.tile([128, D2], f32, tag="h2")
        nc.scalar.activation(h2[:], ps2[:], AF.Tanh)
        prod = sb.tile([128, D2], f32, tag="prod")
        nc.vector.tensor_tensor(prod[:], h2[:], w2b, op=ALU.mult)
        red = sb.tile([128, H], f32, tag="red")
        nc.vector.tensor_reduce(red[:], prod[:].rearrange("p (h d) -> p h d", d=32),
                                axis=mybir.AxisListType.X, op=ALU.add)
        lg = sb.tile([128, H], f32, tag="lg")
        nc.vector.tensor_tensor(lg[:], red[:], b2b, op=ALU.add)
        nc.sync.dma_start(out_d, lg[:])

    nc.compile()
    _CACHE["nc"] = nc
    return nc


def _prep_inputs(x, coef, scale_base, scale_sp, lmd, W1, b1, W2, b2):
    xf = np.asarray(x, np.float64).reshape(B, I)
    coef = np.asarray(coef, np.float64)
    eff = coef * np.asarray(scale_sp, np.float64)[..., None] \
        * np.asarray(lmd, np.float64)[:, :, None, None] / 6.0
    sbl = np.asarray(scale_base, np.float64) \
        * np.asarray(lmd, np.float64)[:, :, None]
    wbig = np.concatenate([eff, sbl[..., None]], -1)            # (H,I,O,9)
    # -> (I, 9, H, O), then piece-major per chunk
    wi = np.ascontiguousarray(wbig.transpose(1, 3, 0, 2)).astype(np.float16)
    pieces = [wi[c * 128:(c + 1) * 128].reshape(-1) for c in range(6)]
    pieces.append(np.ascontiguousarray(wi[768:I, :, 0:5, :]).reshape(-1))
    pieces.append(np.ascontiguousarray(wi[768:I, :, 5:10, :]).reshape(-1))
    wdev = np.concatenate(pieces)

    W1 = np.asarray(W1, np.float64)
    w1bd = np.zeros((HO, D2))
    for h in range(H):
        w1bd[h * O:(h + 1) * O, h * 32:(h + 1) * 32] = W1[h]
    w1dev = np.ascontiguousarray(
        w1bd.reshape(5, 128, D2).transpose(1, 0, 2)).astype(np.float16)
    c16 = np.concatenate([w1dev.reshape(128, 5 * D2),
                          np.eye(128, dtype=np.float16)], 1).astype(np.float16)
    b1c = np.asarray(b1, np.float16).reshape(1, D2).copy()
    c32 = np.ascontiguousarray(np.concatenate([
        np.broadcast_to(np.asarray(W2, np.float32).reshape(D2), (128, D2)),
        np.broadcast_to(np.asarray(b2, np.float32).reshape(H), (128, H))],
        1).astype(np.float32))

    in_maps = []
    for core in range(NC):
        xs = xf[core * BC:(core + 1) * BC].T                     # (784,128)
        xdev = np.zeros((128, CH, BC), np.float32)
        for c in range(CH):
            rows = xs[c * 128:min((c + 1) * 128, I)]
            xdev[0:rows.shape[0], c, :] = rows
        in_maps.append({"x": xdev, "w": wdev, "w1": c16,
                        "b1": b1c, "w2": c32})
    return in_maps


def run(inputs, trace=False, tmpdir=None):
    _install_ntff_hook()
    from concourse.bass_utils import run_bass_kernel_spmd
    nc = _build()
    in_maps = _prep_inputs(**inputs)
    res = run_bass_kernel_spmd(nc, in_maps, core_ids=list(range(NC)),
                               trace=trace, tmpdir=tmpdir)
    out = np.concatenate([r["out"] for r in res.results], 0)
    return out.astype(np.float32), res


def kernel(**inputs):
    out, _ = run(inputs)
    return out



# revision 6
# speedup vs baseline: 1.4067x; 1.4067x over previous
"""Trainium2 Bass kernel for nn_Mnist_lmdSplineKAN.

Sharding: 2D -- batch x4 (256 rows/core) by head-group x2 (5 heads = 320
out cols/core). All 8 cores do identical-shape work.

Math: the uniform-grid cubic B-spline basis is rewritten in the truncated
power basis,  f_j(z) = (1/6) sum_r (-1)^r C(4,r) (z+3-j-r)_+^3  with z=5x.
Splitting each (z-m)_+^3 into a smooth cubic (folded into the weights on
the host) plus a bounded one-sided cube leaves just 8 device feature
planes: d, d^2, d^3 (d = z-2.5), S1=(1-z)_+^3, S2=(2-z)_+^3, R3=(z-3)_+^3,
R4=(z-4)_+^3, and silu(x). The constant term becomes a bias row added via
a rank-1 ones-matmul. Features are fp16 stationary; weights fp16 moving;
PSUM fp32.

I=784 is tiled as 6 full chunks of 128 + 16 leftover rows; the leftover
rows x 8 planes pack into one K=128 matmul via an SBUF->SBUF repack.
Weights stream plane-major on the sync HWDGE queue in matmul consumption
order; warmup matmuls ramp the PE p-state before the real wavefront.
"""
import sys, types
import numpy as np

B, I, O, H = 1024, 784, 64, 10
NC, BG, OG = 8, 4, 2
BC = B // BG          # 256 batch rows per core
HOC = (H // OG) * O   # 320 output cols per core
D2C = (H // OG) * 32  # 160 hidden cols per core
NCH = 6               # full 128-row input chunks
PL = 16               # leftover input rows (chunk 6)
NP = 8                # feature planes
NWARM = 12

# plane order: d, d2, silu, d3, R3, R4, S1, S2
C5 = np.array([1., -4., 6., -4., 1.]) / 6.0


def _tables():
    polyc = np.zeros((8, 4))
    tapS = np.zeros((8, 2))
    tapR = np.zeros((8, 2))
    for j in range(8):
        for r in range(5):
            m = j - 3 + r
            cc = C5[r]
            if m >= 5:
                continue
            if m in (3, 4):
                tapR[j, m - 3] += cc
            else:
                a = 2.5 - m
                polyc[j] += cc * np.array([a**3, 3 * a**2, 3 * a, 1.0])
                if m in (1, 2):
                    tapS[j, m - 1] += cc
    return polyc, tapS, tapR


def _install_ntff_hook():
    if "antenv.axon_hooks" in sys.modules:
        return
    try:
        import antenv
        mod = types.ModuleType("antenv.axon_hooks")
        _h = [None]
        mod.set_axon_ntff_profile_hook = lambda h: _h.__setitem__(0, h)
        mod.get_axon_ntff_profile_hook = lambda: _h[0]
        sys.modules["antenv.axon_hooks"] = mod
        antenv.axon_hooks = mod
        from trn_agent_boot.trn_boot import _ntff_profile_via_ctypes
        h = _ntff_profile_via_ctypes("/opt/axon/libaxon_pjrt.so")
        if h is not None:
            mod.set_axon_ntff_profile_hook(h)
    except Exception:
        pass


_CACHE = {}


def _build():
    if "nc" in _CACHE:
        return _CACHE["nc"]
    import concourse.bacc as bacc
    import concourse.bass as bass
    import concourse.tile as tile
    from concourse import mybir
    from contextlib import ExitStack

    f32, f16 = mybir.dt.float32, mybir.dt.float16
    ALU = mybir.AluOpType
    AF = mybir.ActivationFunctionType

    nc = bacc.Bacc("TRN2", target_bir_lowering=False, debug=False)
    x_d = nc.dram_tensor("x", (128, 7 * BC), f16, kind="ExternalInput").ap()
    WROW = NCH * HOC                       # 1920 elems per partition per plane
    w_d = nc.dram_tensor("w", (NP * 128 * WROW + 128 * HOC,), f16,
                         kind="ExternalInput").ap()
    b_d = nc.dram_tensor("brow", (1, HOC + D2C), f16, kind="ExternalInput").ap()
    cf16_d = nc.dram_tensor("cf16", (128, 3 * D2C + 128), f16,
                            kind="ExternalInput").ap()
    cf32_d = nc.dram_tensor("cf32", (128, D2C + 5), f32,
                            kind="ExternalInput").ap()
    out_d = nc.dram_tensor("out", (BC, 5), f32, kind="ExternalOutput").ap()

    with tile.TileContext(nc) as tc, ExitStack() as ctx:
        sb = ctx.enter_context(tc.tile_pool(name="sb", bufs=1))
        ps = ctx.enter_context(tc.tile_pool(name="ps", bufs=1, space="PSUM"))

        # ---- DMAs: sync HWDGE = bias row, x, weight planes (consumption
        #      order), packed chunk-6; gpsimd SWDGE = tail consts ----
        brow = sb.tile([1, HOC + D2C], f16, tag="brow")
        nc.sync.dma_start(brow[:], b_d)
        xt = sb.tile([128, 7, BC], f16, tag="xt")
        nc.sync.dma_start(xt[:], x_d.rearrange("p (c b) -> p c b", b=BC))
        wAll = sb.tile([128, NP, NCH, HOC], f16, tag="wAll")
        for p in range(NP):
            src = bass.AP(tensor=w_d.tensor, offset=p * 128 * WROW,
                          ap=[[WROW, 128], [1, WROW]])
            nc.sync.dma_start(
                wAll[:, p].rearrange("p c o -> p (c o)"), src)
        w6t = sb.tile([128, HOC], f16, tag="w6t")
        src6 = bass.AP(tensor=w_d.tensor, offset=NP * 128 * WROW,
                       ap=[[HOC, 128], [1, HOC]])
        nc.sync.dma_start(w6t[:], src6)

        cf16 = sb.tile([128, 3 * D2C + 128], f16, tag="cf16")
        nc.gpsimd.dma_start(cf16[:], cf16_d)
        w1p = cf16[:, 0:3 * D2C].rearrange("p (k d) -> p k d", d=D2C)
        idt = cf16[:, 3 * D2C:]
        cf32 = sb.tile([128, D2C + 5], f32, tag="cf32")
        nc.gpsimd.dma_start(cf32[:], cf32_d)
        w2b = cf32[:, 0:D2C]
        b2b = cf32[:, D2C:]

        ones = sb.tile([1, 128], f16, tag="ones")
        nc.vector.memset(ones[:], 1.0)

        # ---- feature planes ----
        fall = sb.tile([128, NP, 7, BC], f16, tag="fall")
        x2 = xt[:].rearrange("p c b -> p (c b)")

        def pl(p):
            return fall[:, p].rearrange("p c b -> p (c b)")

        def T(tag):
            return sb.tile([128, 7 * BC], f16, tag=tag, name=tag)

        # plane order: 0:d 1:d2 2:d3 3:silu 4:R3 5:R4 6:S1 7:S2
        bias2 = sb.tile([128, 1], f32, tag="bias2")
        nc.gpsimd.memset(bias2[:], 2.0)
        s1 = T("s1"); s2 = T("s2"); r3 = T("r3"); r4 = T("r4")
        q1 = T("q1"); q2 = T("q2"); q3 = T("q3"); q4 = T("q4")
        # ACT: d, d2, s1=(1-z)+, silu, s2=(2-z)+, s1^2
        nc.scalar.activation(pl(0), x2, AF.Copy, bias=-2.5, scale=5.0)
        dm = pl(0)
        nc.scalar.activation(pl(1), dm, AF.Square)
        nc.scalar.activation(s1[:], x2, AF.Relu, bias=1.0, scale=-5.0)
        nc.scalar.activation(pl(3), x2, AF.Silu)
        nc.scalar.activation(s2[:], x2, AF.Relu, bias=bias2[:], scale=-5.0)
        nc.scalar.activation(q1[:], s1[:], AF.Square)
        # DVE: r3/r4 relus from d, squares, d3, cubes
        nc.vector.tensor_scalar(r3[:], dm, -0.5, 0.0, op0=ALU.add, op1=ALU.max)
        nc.vector.tensor_scalar(r4[:], dm, -1.5, 0.0, op0=ALU.add, op1=ALU.max)
        nc.vector.tensor_tensor(pl(2), pl(1), dm, op=ALU.mult)
        nc.vector.tensor_tensor(q3[:], r3[:], r3[:], op=ALU.mult)
        nc.vector.tensor_tensor(q4[:], r4[:], r4[:], op=ALU.mult)
        nc.vector.tensor_tensor(pl(4), q3[:], r3[:], op=ALU.mult)
        nc.vector.tensor_tensor(pl(5), q4[:], r4[:], op=ALU.mult)
        nc.vector.tensor_tensor(q2[:], s2[:], s2[:], op=ALU.mult)
        nc.vector.tensor_tensor(pl(6), q1[:], s1[:], op=ALU.mult)
        nc.vector.tensor_tensor(pl(7), q2[:], s2[:], op=ALU.mult)

        # ---- chunk-6 pack: 16 rows x 8 planes -> one K=128 tile ----
        f6 = sb.tile([128, BC], f16, tag="f6")
        for p in range(NP):
            nc.sync.dma_start(f6[p * PL:(p + 1) * PL, :], fall[0:PL, p, 6, :])

        # ---- matmuls ----
        wu = ps.tile([128, HOC], f32, tag="wu")
        for k in range(NWARM):
            nc.tensor.matmul(wu[:], ones[:], brow[0:1, 0:HOC],
                             start=True, stop=True)

        y = [ps.tile([128, HOC], f32, tag=f"y{bt}", name=f"y{bt}")
             for bt in range(2)]
        for bt in range(2):
            nc.tensor.matmul(y[bt][:], ones[:], brow[0:1, 0:HOC],
                             start=True, stop=False)
        for p in range(NP):
            for c in range(NCH):
                for bt in range(2):
                    nc.tensor.matmul(
                        y[bt][:], fall[:, p, c, bt * 128:(bt + 1) * 128],
                        wAll[:, p, c, :], start=False, stop=False)
        for bt in range(2):
            nc.tensor.matmul(y[bt][:], f6[:, bt * 128:(bt + 1) * 128],
                             w6t[:], start=False, stop=True)

        # ---- tail per batch-tile: tanh, transpose, blockdiag MLP ----
        for bt in range(2):
            h1 = sb.tile([128, HOC], f16, tag=f"h1{bt}", name=f"h1{bt}")
            nc.scalar.activation(h1[:], y[bt][:], AF.Tanh)
            sts = []
            for k in range(3):
                kk = 128 if k < 2 else 64
                pt = ps.tile([128, 128], f16, tag=f"pt{k}",
                             name=f"pt{bt}{k}")
                nc.tensor.transpose(pt[0:kk, :], h1[:, k * 128:k * 128 + kk],
                                    idt)
                st = sb.tile([128, 128], f16, tag=f"st{bt}{k}",
                             name=f"st{bt}{k}")
                nc.vector.tensor_copy(st[0:kk, :], pt[0:kk, :])
                sts.append(st)
            ps2 = ps.tile([128, D2C], f32, tag=f"ps2{bt}", name=f"ps2{bt}")
            nc.tensor.matmul(ps2[:], ones[:], brow[0:1, HOC:],
                             start=True, stop=False)
            for k in range(3):
                kk = 128 if k < 2 else 64
                nc.tensor.matmul(ps2[:], sts[k][0:kk, :], w1p[0:kk, k, :],
                                 start=False, stop=(k == 2))
            h2 = sb.tile([128, D2C], f32, tag=f"h2{bt}", name=f"h2{bt}")
            nc.scalar.activation(h2[:], ps2[:], AF.Tanh)
            prod = sb.tile([128, D2C], f32, tag=f"prod{bt}", name=f"prod{bt}")
            nc.vector.tensor_tensor(prod[:], h2[:], w2b, op=ALU.mult)
            red = sb.tile([128, 5], f32, tag=f"red{bt}", name=f"red{bt}")
            nc.vector.tensor_reduce(
                red[:], prod[:].rearrange("p (h d) -> p h d", d=32),
                axis=mybir.AxisListType.X, op=ALU.add)
            lg = sb.tile([128, 5], f32, tag=f"lg{bt}", name=f"lg{bt}")
            nc.vector.tensor_tensor(lg[:], red[:], b2b, op=ALU.add)
            nc.sync.dma_start(out_d[bt * 128:(bt + 1) * 128, :], lg[:])

    nc.compile()
    _CACHE["nc"] = nc
    return nc


def _prep_inputs(x, coef, scale_base, scale_sp, lmd, W1, b1, W2, b2):
    polyc, tapS, tapR = _tables()
    xf = np.asarray(x, np.float32).reshape(B, I)

    coef = np.asarray(coef, np.float64)
    eff = coef * np.asarray(scale_sp, np.float64)[..., None] \
        * np.asarray(lmd, np.float64)[:, :, None, None]        # (H, I, O, 8)
    W = eff.transpose(1, 3, 0, 2).reshape(I, 8, H * O)         # (I, 8, 640)
    sbl = (np.asarray(scale_base, np.float64)
           * np.asarray(lmd, np.float64)[:, :, None]
           ).transpose(1, 0, 2).reshape(I, H * O)

    # fold: plane order d, d2, d3, silu, R3, R4, S1, S2
    Wp = np.empty((I, NP, H * O))
    Wp[:, 0] = np.einsum('j,ijo->io', polyc[:, 1], W)
    Wp[:, 1] = np.einsum('j,ijo->io', polyc[:, 2], W)
    Wp[:, 2] = np.einsum('j,ijo->io', polyc[:, 3], W)
    Wp[:, 3] = sbl
    Wp[:, 4] = np.einsum('j,ijo->io', tapR[:, 0], W)
    Wp[:, 5] = np.einsum('j,ijo->io', tapR[:, 1], W)
    Wp[:, 6] = np.einsum('j,ijo->io', tapS[:, 0], W)
    Wp[:, 7] = np.einsum('j,ijo->io', tapS[:, 1], W)
    bias_full = np.einsum('j,ijo->o', polyc[:, 0], W)          # (640,)

    W1 = np.asarray(W1, np.float64)
    W2 = np.asarray(W2, np.float64).reshape(H * 32)
    b1 = np.asarray(b1, np.float64).reshape(H * 32)
    b2 = np.asarray(b2, np.float64).reshape(H)

    per_og = []
    for og in range(OG):
        hs = slice(og * HOC, (og + 1) * HOC)
        # weight stream: 8 plane pieces [128, 6*320] then packed chunk-6
        pieces = []
        for p in range(NP):
            blk = Wp[0:NCH * 128, p, hs].reshape(NCH, 128, HOC)
            pieces.append(np.ascontiguousarray(
                blk.transpose(1, 0, 2)).reshape(-1))
        w6 = np.zeros((128, HOC))
        for p in range(NP):
            w6[p * PL:(p + 1) * PL] = Wp[NCH * 128:I, p, hs]
        pieces.append(np.ascontiguousarray(w6).reshape(-1))
        wdev = np.concatenate(pieces).astype(np.float16)

        brow = np.zeros((1, HOC + D2C))
        brow[0, 0:HOC] = bias_full[hs]
        brow[0, HOC:] = b1[og * D2C:(og + 1) * D2C]
        brow = brow.astype(np.float16)

        w1bd = np.zeros((HOC, D2C))
        for hl in range(H // OG):
            w1bd[hl * O:(hl + 1) * O, hl * 32:(hl + 1) * 32] = W1[og * (H // OG) + hl]
        w1dev = np.zeros((128, 3, D2C))
        w1dev[:, 0] = w1bd[0:128]
        w1dev[:, 1] = w1bd[128:256]
        w1dev[0:64, 2] = w1bd[256:HOC]
        cf16 = np.concatenate([w1dev.reshape(128, 3 * D2C),
                               np.eye(128)], 1).astype(np.float16)
        cf32 = np.concatenate([
            np.broadcast_to(W2[og * D2C:(og + 1) * D2C], (128, D2C)),
            np.broadcast_to(b2[og * 5:(og + 1) * 5], (128, 5))],
            1).astype(np.float32)
        per_og.append((wdev, brow, cf16, cf32))

    in_maps = []
    for core in range(NC):
        bg, og = core % BG, core // BG
        xs = xf[bg * BC:(bg + 1) * BC].T.astype(np.float16)    # (784, 256)
        xdev = np.zeros((7, 128, BC), np.float16)
        xdev.reshape(7 * 128, BC)[0:I] = xs
        xdev = np.ascontiguousarray(xdev.transpose(1, 0, 2)).reshape(128, 7 * BC)
        wdev, brow, cf16, cf32 = per_og[og]
        in_maps.append({"x": xdev, "w": wdev, "brow": brow,
                        "cf16": cf16, "cf32": cf32})
    return in_maps


def run(inputs, trace=False, tmpdir=None):
    _install_ntff_hook()
    from concourse.bass_utils import run_bass_kernel_spmd
    nc = _build()
    in_maps = _prep_inputs(**inputs)
    res = run_bass_kernel_spmd(nc, in_maps, core_ids=list(range(NC)),
                               trace=trace, tmpdir=tmpdir)
    out = np.empty((B, H), np.float32)
    for core in range(NC):
        bg, og = core % BG, core // BG
        out[bg * BC:(bg + 1) * BC, og * 5:(og + 1) * 5] = res.results[core]["out"]
    return out, res


def kernel(**inputs):
    out, _ = run(inputs)
    return out


# revision 8
# speedup vs baseline: 1.4221x; 1.0110x over previous
"""Trainium2 Bass kernel for nn_Mnist_lmdSplineKAN.

Sharding: 2D -- batch x4 (256 rows/core) by head-group x2 (5 heads = 320
out cols/core). All 8 cores do identical-shape work.

Math: the uniform-grid cubic B-spline basis is rewritten in the truncated
power basis,  f_j(z) = (1/6) sum_r (-1)^r C(4,r) (z+3-j-r)_+^3  with z=5x.
Splitting each (z-m)_+^3 into a smooth cubic (folded into the weights on
the host) plus a bounded one-sided cube leaves just 8 device feature
planes: d, d^2, d^3 (d = z-2.5), S1=(1-z)_+^3, S2=(2-z)_+^3, R3=(z-3)_+^3,
R4=(z-4)_+^3, and silu(x). The constant term becomes a bias row added via
a rank-1 ones-matmul. Features are fp16 stationary; weights fp16 moving;
PSUM fp32.

I=784 is tiled as 6 full chunks of 128 + 16 leftover rows; the leftover
rows x 8 planes pack into one K=128 matmul via an SBUF->SBUF repack.
Weights stream plane-major on the sync HWDGE queue in matmul consumption
order; warmup matmuls ramp the PE p-state before the real wavefront.
"""
import sys, types
import numpy as np

B, I, O, H = 1024, 784, 64, 10
NC, BG, OG = 8, 4, 2
BC = B // BG          # 256 batch rows per core
HOC = (H // OG) * O   # 320 output cols per core
D2C = (H // OG) * 32  # 160 hidden cols per core
NCH = 6               # full 128-row input chunks
PL = 16               # leftover input rows (chunk 6)
NP = 8                # feature planes
NWARM = 12

# plane order: d, d2, silu, d3, R3, R4, S1, S2
C5 = np.array([1., -4., 6., -4., 1.]) / 6.0


def _tables():
    polyc = np.zeros((8, 4))
    tapS = np.zeros((8, 2))
    tapR = np.zeros((8, 2))
    for j in range(8):
        for r in range(5):
            m = j - 3 + r
            cc = C5[r]
            if m >= 5:
                continue
            if m in (3, 4):
                tapR[j, m - 3] += cc
            else:
                a = 2.5 - m
                polyc[j] += cc * np.array([a**3, 3 * a**2, 3 * a, 1.0])
                if m in (1, 2):
                    tapS[j, m - 1] += cc
    return polyc, tapS, tapR


def _install_ntff_hook():
    if "antenv.axon_hooks" in sys.modules:
        return
    try:
        import antenv
        mod = types.ModuleType("antenv.axon_hooks")
        _h = [None]
        mod.set_axon_ntff_profile_hook = lambda h: _h.__setitem__(0, h)
        mod.get_axon_ntff_profile_hook = lambda: _h[0]
        sys.modules["antenv.axon_hooks"] = mod
        antenv.axon_hooks = mod
        from trn_agent_boot.trn_boot import _ntff_profile_via_ctypes
        h = _ntff_profile_via_ctypes("/opt/axon/libaxon_pjrt.so")
        if h is not None:
            mod.set_axon_ntff_profile_hook(h)
    except Exception:
        pass


_CACHE = {}


def _build():
    if "nc" in _CACHE:
        return _CACHE["nc"]
    import concourse.bacc as bacc
    import concourse.bass as bass
    import concourse.tile as tile
    from concourse import mybir
    from contextlib import ExitStack

    f32, f16 = mybir.dt.float32, mybir.dt.float16
    ALU = mybir.AluOpType
    AF = mybir.ActivationFunctionType

    nc = bacc.Bacc("TRN2", target_bir_lowering=False, debug=False)
    x_d = nc.dram_tensor("x", (128, 7 * BC), f16, kind="ExternalInput").ap()
    WROW = NCH * HOC                       # 1920 elems per partition per plane
    w_d = nc.dram_tensor("w", (NP * 128 * WROW + 128 * HOC,), f16,
                         kind="ExternalInput").ap()
    b_d = nc.dram_tensor("brow", (1, HOC + D2C), f16, kind="ExternalInput").ap()
    cf16_d = nc.dram_tensor("cf16", (128, 3 * D2C + 128), f16,
                            kind="ExternalInput").ap()
    cf32_d = nc.dram_tensor("cf32", (128, D2C + 5), f32,
                            kind="ExternalInput").ap()
    out_d = nc.dram_tensor("out", (BC, 5), f32, kind="ExternalOutput").ap()

    with tile.TileContext(nc) as tc, ExitStack() as ctx:
        sb = ctx.enter_context(tc.tile_pool(name="sb", bufs=1))
        ps = ctx.enter_context(tc.tile_pool(name="ps", bufs=1, space="PSUM"))

        # ---- DMAs: sync HWDGE = bias row, x, weight planes (consumption
        #      order), packed chunk-6; gpsimd SWDGE = tail consts ----
        brow = sb.tile([1, HOC + D2C], f16, tag="brow")
        nc.sync.dma_start(brow[:], b_d)
        xt = sb.tile([128, 7, BC], f16, tag="xt")
        nc.sync.dma_start(xt[:], x_d.rearrange("p (c b) -> p c b", b=BC))
        wAll = sb.tile([128, NP, NCH, HOC], f16, tag="wAll")
        for p in range(NP):
            src = bass.AP(tensor=w_d.tensor, offset=p * 128 * WROW,
                          ap=[[WROW, 128], [1, WROW]])
            nc.gpsimd.dma_start(
                wAll[:, p].rearrange("p c o -> p (c o)"), src)
        w6t = sb.tile([128, HOC], f16, tag="w6t")
        src6 = bass.AP(tensor=w_d.tensor, offset=NP * 128 * WROW,
                       ap=[[HOC, 128], [1, HOC]])
        nc.gpsimd.dma_start(w6t[:], src6)

        cf16 = sb.tile([128, 3 * D2C + 128], f16, tag="cf16")
        nc.gpsimd.dma_start(cf16[:], cf16_d)
        w1p = cf16[:, 0:3 * D2C].rearrange("p (k d) -> p k d", d=D2C)
        idt = cf16[:, 3 * D2C:]
        cf32 = sb.tile([128, D2C + 5], f32, tag="cf32")
        nc.gpsimd.dma_start(cf32[:], cf32_d)
        w2b = cf32[:, 0:D2C]
        b2b = cf32[:, D2C:]

        ones = sb.tile([1, 128], f16, tag="ones")
        nc.vector.memset(ones[:], 1.0)

        # force both ACT tables to load during the DMA-wait window
        tl = sb.tile([1, 4], f16, tag="tl")
        for fn in (AF.Copy, AF.Square, AF.Relu, AF.Silu, AF.Tanh):
            nc.scalar.activation(tl[0:1, 0:1], ones[0:1, 0:1], fn)

        # ---- feature planes ----
        fall = sb.tile([128, NP, 7, BC], f16, tag="fall")
        x2 = xt[:].rearrange("p c b -> p (c b)")

        def pl(p):
            return fall[:, p].rearrange("p c b -> p (c b)")

        def T(tag):
            return sb.tile([128, 7 * BC], f16, tag=tag, name=tag)

        # plane order: 0:d 1:d2 2:d3 3:silu 4:R3 5:R4 6:S1 7:S2
        bias2 = sb.tile([128, 1], f32, tag="bias2")
        nc.gpsimd.memset(bias2[:], 2.0)
        s1 = T("s1"); s2 = T("s2"); r3 = T("r3"); r4 = T("r4")
        q1 = T("q1"); q2 = T("q2"); q3 = T("q3"); q4 = T("q4")
        # ACT: d, d2, s1=(1-z)+, silu, s2=(2-z)+, s1^2
        nc.scalar.activation(pl(0), x2, AF.Copy, bias=-2.5, scale=5.0)
        dm = pl(0)
        nc.scalar.activation(pl(1), dm, AF.Square)
        nc.scalar.activation(s1[:], x2, AF.Relu, bias=1.0, scale=-5.0)
        nc.scalar.activation(pl(3), x2, AF.Silu)
        nc.scalar.activation(s2[:], x2, AF.Relu, bias=bias2[:], scale=-5.0)
        nc.scalar.activation(q1[:], s1[:], AF.Square)
        # DVE: r3/r4 relus from d, squares, d3, cubes
        nc.vector.tensor_scalar(r3[:], dm, -0.5, 0.0, op0=ALU.add, op1=ALU.max)
        nc.vector.tensor_scalar(r4[:], dm, -1.5, 0.0, op0=ALU.add, op1=ALU.max)
        nc.vector.tensor_tensor(pl(2), pl(1), dm, op=ALU.mult)
        nc.vector.tensor_tensor(q3[:], r3[:], r3[:], op=ALU.mult)
        nc.vector.tensor_tensor(q4[:], r4[:], r4[:], op=ALU.mult)
        nc.vector.tensor_tensor(pl(4), q3[:], r3[:], op=ALU.mult)
        nc.vector.tensor_tensor(pl(5), q4[:], r4[:], op=ALU.mult)
        nc.vector.tensor_tensor(q2[:], s2[:], s2[:], op=ALU.mult)
        nc.vector.tensor_tensor(pl(6), q1[:], s1[:], op=ALU.mult)
        nc.vector.tensor_tensor(pl(7), q2[:], s2[:], op=ALU.mult)

        # ---- chunk-6 pack: 16 rows x 8 planes -> one K=128 tile ----
        f6 = sb.tile([128, BC], f16, tag="f6")
        for p in range(NP):
            nc.sync.dma_start(f6[p * PL:(p + 1) * PL, :], fall[0:PL, p, 6, :])

        # ---- matmuls ----
        wu = ps.tile([128, HOC], f32, tag="wu")
        for k in range(NWARM):
            nc.tensor.matmul(wu[:], ones[:], brow[0:1, 0:HOC],
                             start=True, stop=True)

        y = [ps.tile([128, HOC], f32, tag=f"y{bt}", name=f"y{bt}")
             for bt in range(2)]
        for bt in range(2):
            nc.tensor.matmul(y[bt][:], ones[:], brow[0:1, 0:HOC],
                             start=True, stop=False)
        for p in range(NP):
            for c in range(NCH):
                for bt in range(2):
                    nc.tensor.matmul(
                        y[bt][:], fall[:, p, c, bt * 128:(bt + 1) * 128],
                        wAll[:, p, c, :], start=False, stop=False)
        for bt in range(2):
            nc.tensor.matmul(y[bt][:], f6[:, bt * 128:(bt + 1) * 128],
                             w6t[:], start=False, stop=True)

        # ---- tail per batch-tile: tanh, transpose, blockdiag MLP ----
        for bt in range(2):
            h1 = sb.tile([128, HOC], f16, tag=f"h1{bt}", name=f"h1{bt}")
            nc.scalar.activation(h1[:], y[bt][:], AF.Tanh)
            sts = []
            for k in range(3):
                kk = 128 if k < 2 else 64
                pt = ps.tile([128, 128], f16, tag=f"pt{k}",
                             name=f"pt{bt}{k}")
                nc.tensor.transpose(pt[0:kk, :], h1[:, k * 128:k * 128 + kk],
                                    idt)
                st = sb.tile([128, 128], f16, tag=f"st{bt}{k}",
                             name=f"st{bt}{k}")
                nc.vector.tensor_copy(st[0:kk, :], pt[0:kk, :])
                sts.append(st)
            ps2 = ps.tile([128, D2C], f32, tag=f"ps2{bt}", name=f"ps2{bt}")
            nc.tensor.matmul(ps2[:], ones[:], brow[0:1, HOC:],
                             start=True, stop=False)
            for k in range(3):
                kk = 128 if k < 2 else 64
                nc.tensor.matmul(ps2[:], sts[k][0:kk, :], w1p[0:kk, k, :],
                                 start=False, stop=(k == 2))
            h2 = sb.tile([128, D2C], f32, tag=f"h2{bt}", name=f"h2{bt}")
            nc.scalar.activation(h2[:], ps2[:], AF.Tanh)
            prod = sb.tile([128, D2C], f32, tag=f"prod{bt}", name=f"prod{bt}")
            nc.vector.tensor_tensor(prod[:], h2[:], w2b, op=ALU.mult)
            red = sb.tile([128, 5], f32, tag=f"red{bt}", name=f"red{bt}")
            nc.vector.tensor_reduce(
                red[:], prod[:].rearrange("p (h d) -> p h d", d=32),
                axis=mybir.AxisListType.X, op=ALU.add)
            lg = sb.tile([128, 5], f32, tag=f"lg{bt}", name=f"lg{bt}")
            nc.vector.tensor_tensor(lg[:], red[:], b2b, op=ALU.add)
            nc.sync.dma_start(out_d[bt * 128:(bt + 1) * 128, :], lg[:])

    nc.compile()
    _CACHE["nc"] = nc
    return nc


def _prep_inputs(x, coef, scale_base, scale_sp, lmd, W1, b1, W2, b2):
    polyc, tapS, tapR = _tables()
    xf = np.asarray(x, np.float32).reshape(B, I)

    coef = np.asarray(coef, np.float64)
    eff = coef * np.asarray(scale_sp, np.float64)[..., None] \
        * np.asarray(lmd, np.float64)[:, :, None, None]        # (H, I, O, 8)
    W = eff.transpose(1, 3, 0, 2).reshape(I, 8, H * O)         # (I, 8, 640)
    sbl = (np.asarray(scale_base, np.float64)
           * np.asarray(lmd, np.float64)[:, :, None]
           ).transpose(1, 0, 2).reshape(I, H * O)

    # fold: plane order d, d2, d3, silu, R3, R4, S1, S2
    Wp = np.empty((I, NP, H * O))
    Wp[:, 0] = np.einsum('j,ijo->io', polyc[:, 1], W)
    Wp[:, 1] = np.einsum('j,ijo->io', polyc[:, 2], W)
    Wp[:, 2] = np.einsum('j,ijo->io', polyc[:, 3], W)
    Wp[:, 3] = sbl
    Wp[:, 4] = np.einsum('j,ijo->io', tapR[:, 0], W)
    Wp[:, 5] = np.einsum('j,ijo->io', tapR[:, 1], W)
    Wp[:, 6] = np.einsum('j,ijo->io', tapS[:, 0], W)
    Wp[:, 7] = np.einsum('j,ijo->io', tapS[:, 1], W)
    bias_full = np.einsum('j,ijo->o', polyc[:, 0], W)          # (640,)

    W1 = np.asarray(W1, np.float64)
    W2 = np.asarray(W2, np.float64).reshape(H * 32)
    b1 = np.asarray(b1, np.float64).reshape(H * 32)
    b2 = np.asarray(b2, np.float64).reshape(H)

    per_og = []
    for og in range(OG):
        hs = slice(og * HOC, (og + 1) * HOC)
        # weight stream: 8 plane pieces [128, 6*320] then packed chunk-6
        pieces = []
        for p in range(NP):
            blk = Wp[0:NCH * 128, p, hs].reshape(NCH, 128, HOC)
            pieces.append(np.ascontiguousarray(
                blk.transpose(1, 0, 2)).reshape(-1))
        w6 = np.zeros((128, HOC))
        for p in range(NP):
            w6[p * PL:(p + 1) * PL] = Wp[NCH * 128:I, p, hs]
        pieces.append(np.ascontiguousarray(w6).reshape(-1))
        wdev = np.concatenate(pieces).astype(np.float16)

        brow = np.zeros((1, HOC + D2C))
        brow[0, 0:HOC] = bias_full[hs]
        brow[0, HOC:] = b1[og * D2C:(og + 1) * D2C]
        brow = brow.astype(np.float16)

        w1bd = np.zeros((HOC, D2C))
        for hl in range(H // OG):
            w1bd[hl * O:(hl + 1) * O, hl * 32:(hl + 1) * 32] = W1[og * (H // OG) + hl]
        w1dev = np.zeros((128, 3, D2C))
        w1dev[:, 0] = w1bd[0:128]
        w1dev[:, 1] = w1bd[128:256]
        w1dev[0:64, 2] = w1bd[256:HOC]
        cf16 = np.concatenate([w1dev.reshape(128, 3 * D2C),
                               np.eye(128)], 1).astype(np.float16)
        cf32 = np.concatenate([
            np.broadcast_to(W2[og * D2C:(og + 1) * D2C], (128, D2C)),
            np.broadcast_to(b2[og * 5:(og + 1) * 5], (128, 5))],
            1).astype(np.float32)
        per_og.append((wdev, brow, cf16, cf32))

    in_maps = []
    for core in range(NC):
        bg, og = core % BG, core // BG
        xs = xf[bg * BC:(bg + 1) * BC].T.astype(np.float16)    # (784, 256)
        xdev = np.zeros((7, 128, BC), np.float16)
        xdev.reshape(7 * 128, BC)[0:I] = xs
        xdev = np.ascontiguousarray(xdev.transpose(1, 0, 2)).reshape(128, 7 * BC)
        wdev, brow, cf16, cf32 = per_og[og]
        in_maps.append({"x": xdev, "w": wdev, "brow": brow,
                        "cf16": cf16, "cf32": cf32})
    return in_maps


def run(inputs, trace=False, tmpdir=None):
    _install_ntff_hook()
    from concourse.bass_utils import run_bass_kernel_spmd
    nc = _build()
    in_maps = _prep_inputs(**inputs)
    res = run_bass_kernel_spmd(nc, in_maps, core_ids=list(range(NC)),
                               trace=trace, tmpdir=tmpdir)
    out = np.empty((B, H), np.float32)
    for core in range(NC):
        bg, og = core % BG, core // BG
        out[bg * BC:(bg + 1) * BC, og * 5:(og + 1) * 5] = res.results[core]["out"]
    return out, res


def kernel(**inputs):
    out, _ = run(inputs)
    return out


# revision 19
# speedup vs baseline: 1.5152x; 1.0655x over previous
"""Trainium2 Bass kernel for nn_Mnist_lmdSplineKAN.

Sharding: 2D -- batch x4 (256 rows/core) by head-group x2 (5 heads = 320
out cols/core). All 8 cores do identical-shape work.

Math: the uniform-grid cubic B-spline basis is rewritten in the truncated
power basis,  f_j(z) = (1/6) sum_r (-1)^r C(4,r) (z+3-j-r)_+^3  with z=5x.
Splitting each (z-m)_+^3 into a smooth cubic (folded into the weights on
the host) plus a bounded one-sided cube leaves just 8 device feature
planes: d, d^2, d^3 (d = z-2.5), S1=(1-z)_+^3, S2=(2-z)_+^3, R3=(z-3)_+^3,
R4=(z-4)_+^3, and silu(x). The constant term becomes a bias row added via
a rank-1 ones-matmul. Features are fp16 stationary; weights fp16 moving;
PSUM fp32.

I=784 is tiled as 6 full chunks of 128 + 16 leftover rows; the leftover
rows x 8 planes pack into one K=128 matmul via an SBUF->SBUF repack.
Weights stream plane-major on the sync HWDGE queue in matmul consumption
order; warmup matmuls ramp the PE p-state before the real wavefront.
"""
import sys, types
import numpy as np

B, I, O, H = 1024, 784, 64, 10
NC, BG, OG = 8, 4, 2
BC = B // BG          # 256 batch rows per core
HOC = (H // OG) * O   # 320 output cols per core
D2C = (H // OG) * 32  # 160 hidden cols per core
NCH = 6               # full 128-row input chunks
PL = 16               # leftover input rows (chunk 6)
NP = 7                # feature planes: xc, xc^2, xc^3, R3, R4, S1, S2
NWARM = 14

# plane order: d, d2, silu, d3, R3, R4, S1, S2
C5 = np.array([1., -4., 6., -4., 1.]) / 6.0


def _tables():
    polyc = np.zeros((8, 4))
    tapS = np.zeros((8, 2))
    tapR = np.zeros((8, 2))
    for j in range(8):
        for r in range(5):
            m = j - 3 + r
            cc = C5[r]
            if m >= 5:
                continue
            if m in (3, 4):
                tapR[j, m - 3] += cc
            else:
                a = 2.5 - m
                polyc[j] += cc * np.array([a**3, 3 * a**2, 3 * a, 1.0])
                if m in (1, 2):
                    tapS[j, m - 1] += cc
    return polyc, tapS, tapR


def _install_ntff_hook():
    if "antenv.axon_hooks" in sys.modules:
        return
    try:
        import antenv
        mod = types.ModuleType("antenv.axon_hooks")
        _h = [None]
        mod.set_axon_ntff_profile_hook = lambda h: _h.__setitem__(0, h)
        mod.get_axon_ntff_profile_hook = lambda: _h[0]
        sys.modules["antenv.axon_hooks"] = mod
        antenv.axon_hooks = mod
        from trn_agent_boot.trn_boot import _ntff_profile_via_ctypes
        h = _ntff_profile_via_ctypes("/opt/axon/libaxon_pjrt.so")
        if h is not None:
            mod.set_axon_ntff_profile_hook(h)
    except Exception:
        pass


_CACHE = {}


def _build():
    if "nc" in _CACHE:
        return _CACHE["nc"]
    import concourse.bacc as bacc
    import concourse.bass as bass
    import concourse.tile as tile
    from concourse import mybir
    from contextlib import ExitStack

    f32, f16 = mybir.dt.float32, mybir.dt.float16
    ALU = mybir.AluOpType
    AF = mybir.ActivationFunctionType

    nc = bacc.Bacc("TRN2", target_bir_lowering=False, debug=False)
    x_d = nc.dram_tensor("x", (128, 7 * BC), f16, kind="ExternalInput").ap()
    WROW = NCH * HOC                       # 1920 elems per partition per plane
    w_d = nc.dram_tensor("w", (NP * 128 * WROW + NP * PL * HOC,), f16,
                         kind="ExternalInput").ap()
    b_d = nc.dram_tensor("brow", (1, HOC + D2C), f16, kind="ExternalInput").ap()
    cf16_d = nc.dram_tensor("cf16", (128, 3 * D2C + 128), f16,
                            kind="ExternalInput").ap()
    cf32_d = nc.dram_tensor("cf32", (128, D2C + 5), f32,
                            kind="ExternalInput").ap()
    out_d = nc.dram_tensor("out", (BC, 5), f32, kind="ExternalOutput").ap()

    with tile.TileContext(nc) as tc, ExitStack() as ctx:
        sb = ctx.enter_context(tc.tile_pool(name="sb", bufs=1))
        ps = ctx.enter_context(tc.tile_pool(name="ps", bufs=1, space="PSUM"))

        # ---- DMAs: sync HWDGE = bias row + x (features depend on x);
        #      gpsimd SWDGE = weight planes in consumption order, then
        #      packed chunk-6 and tail consts ----
        brow = sb.tile([1, HOC + D2C], f16, tag="brow")
        nc.sync.dma_start(brow[:], b_d)
        xt = sb.tile([128, 7, BC], f16, tag="xt")
        nc.sync.dma_start(xt[:], x_d.rearrange("p (c b) -> p c b", b=BC))
        wAll = sb.tile([128, NP, NCH, HOC], f16, tag="wAll")
        for p in range(NP):
            src = bass.AP(tensor=w_d.tensor, offset=p * 128 * WROW,
                          ap=[[WROW, 128], [1, WROW]])
            nc.gpsimd.dma_start(
                wAll[:, p].rearrange("p c o -> p (c o)"), src)
        w6t = sb.tile([NP * PL, HOC], f16, tag="w6t")
        src6 = bass.AP(tensor=w_d.tensor, offset=NP * 128 * WROW,
                       ap=[[HOC, NP * PL], [1, HOC]])
        nc.gpsimd.dma_start(w6t[:], src6)

        cf16 = sb.tile([128, 3 * D2C + 128], f16, tag="cf16")
        nc.gpsimd.dma_start(cf16[:], cf16_d)
        w1p = cf16[:, 0:3 * D2C].rearrange("p (k d) -> p k d", d=D2C)
        idt = cf16[:, 3 * D2C:]
        cf32 = sb.tile([128, D2C + 5], f32, tag="cf32")
        nc.gpsimd.dma_start(cf32[:], cf32_d)
        w2b = cf32[:, 0:D2C]
        b2b = cf32[:, D2C:]

        ones = sb.tile([1, 128], f16, tag="ones")
        nc.vector.memset(ones[:], 1.0)

        # force ACT tables to load during the DMA-wait window
        tl = sb.tile([1, 4], f16, tag="tl")
        for fn in (AF.Square, AF.Relu, AF.Tanh):
            nc.scalar.activation(tl[0:1, 0:1], ones[0:1, 0:1], fn)

        # ---- feature planes; xc = x - 0.5 comes pre-centered from host ----
        # plane order: 0:xc 1:xc^2 2:xc^3 3:R3 4:R4 5:S1 6:S2 where
        # R3=(x-0.6)+^3, R4=(x-0.8)+^3, S1=(0.2-x)+^3, S2=(0.4-x)+^3
        fall = sb.tile([128, NP - 1, 7, BC], f16, tag="fall")
        x2 = xt[:].rearrange("p c b -> p (c b)")

        def pl(p):
            if p == 0:
                return x2
            return fall[:, p - 1].rearrange("p c b -> p (c b)")

        def plc(p, c, bt):
            if p == 0:
                return xt[:, c, bt * 128:(bt + 1) * 128]
            return fall[:, p - 1, c, bt * 128:(bt + 1) * 128]

        def T(tag):
            return sb.tile([128, 7 * BC], f16, tag=tag, name=tag)

        bm3 = sb.tile([128, 1], f32, tag="bm3")
        nc.gpsimd.memset(bm3[:], -0.3)
        bm1 = sb.tile([128, 1], f32, tag="bm1")
        nc.gpsimd.memset(bm1[:], -0.1)
        s1 = T("s1"); s2 = T("s2"); r3 = T("r3"); r4 = T("r4")
        q1 = T("q1"); q2 = T("q2"); q3 = T("q3"); q4 = T("q4")
        # ACT: xc^2, s1=(0.2-x)+, s2=(0.4-x)+, s2^2
        nc.scalar.activation(pl(1), x2, AF.Square)
        nc.scalar.activation(s1[:], x2, AF.Relu, bias=bm3[:], scale=-1.0)
        nc.scalar.activation(s2[:], x2, AF.Relu, bias=bm1[:], scale=-1.0)
        nc.scalar.activation(q2[:], s2[:], AF.Square)
        # DVE: r3/r4 relus, xc^3, squares, cubes
        nc.vector.tensor_scalar(r3[:], x2, -0.1, 0.0, op0=ALU.add, op1=ALU.max)
        nc.vector.tensor_scalar(r4[:], x2, -0.3, 0.0, op0=ALU.add, op1=ALU.max)
        nc.vector.tensor_tensor(pl(2), pl(1), x2, op=ALU.mult)
        nc.vector.tensor_tensor(q3[:], r3[:], r3[:], op=ALU.mult)
        nc.vector.tensor_tensor(q4[:], r4[:], r4[:], op=ALU.mult)
        nc.vector.tensor_tensor(pl(3), q3[:], r3[:], op=ALU.mult)
        nc.vector.tensor_tensor(pl(4), q4[:], r4[:], op=ALU.mult)
        nc.vector.tensor_tensor(q1[:], s1[:], s1[:], op=ALU.mult)
        nc.vector.tensor_tensor(pl(5), q1[:], s1[:], op=ALU.mult)
        nc.vector.tensor_tensor(pl(6), q2[:], s2[:], op=ALU.mult)

        # ---- chunk-6 pack: 16 rows x 7 planes -> one K=112 tile ----
        f6 = sb.tile([NP * PL, BC], f16, tag="f6")
        nc.sync.dma_start(f6[0:PL, :], xt[0:PL, 6, :])
        for p in range(1, NP):
            nc.sync.dma_start(f6[p * PL:(p + 1) * PL, :],
                              fall[0:PL, p - 1, 6, :])

        # ---- matmuls ----
        wu = ps.tile([128, HOC], f32, tag="wu")
        for k in range(NWARM):
            nc.tensor.matmul(wu[:], ones[:], brow[0:1, 0:HOC],
                             start=True, stop=True)

        y = [ps.tile([128, HOC], f32, tag=f"y{bt}", name=f"y{bt}")
             for bt in range(2)]
        for bt in range(2):
            nc.tensor.matmul(y[bt][:], ones[:], brow[0:1, 0:HOC],
                             start=True, stop=False)
        for p in range(NP - 1):
            for c in range(NCH):
                for bt in range(2):
                    nc.tensor.matmul(y[bt][:], plc(p, c, bt),
                                     wAll[:, p, c, :], start=False, stop=False)
        # last plane + packed chunk-6: all of bt0 first so its PSUM bank
        # closes early and the bt0 tail overlaps bt1's matmuls
        for bt in range(2):
            for c in range(NCH):
                nc.tensor.matmul(y[bt][:], plc(NP - 1, c, bt),
                                 wAll[:, NP - 1, c, :], start=False, stop=False)
            nc.tensor.matmul(y[bt][:], f6[:, bt * 128:(bt + 1) * 128],
                             w6t[:], start=False, stop=True)

        # ---- tail per batch-tile: tanh, transpose, blockdiag MLP ----
        lgs = sb.tile([128, 2, 5], f32, tag="lgs")
        for bt in range(2):
            h1 = sb.tile([128, HOC], f16, tag=f"h1{bt}", name=f"h1{bt}")
            nc.scalar.activation(h1[:], y[bt][:], AF.Tanh)
            sts = []
            for k in range(3):
                kk = 128 if k < 2 else 64
                pt = ps.tile([128, 128], f16, tag=f"pt{k}",
                             name=f"pt{bt}{k}")
                nc.tensor.transpose(pt[0:kk, :], h1[:, k * 128:k * 128 + kk],
                                    idt)
                st = sb.tile([128, 128], f16, tag=f"st{bt}{k}",
                             name=f"st{bt}{k}")
                nc.vector.tensor_copy(st[0:kk, :], pt[0:kk, :])
                sts.append(st)
            ps2 = ps.tile([128, D2C], f32, tag=f"ps2{bt}", name=f"ps2{bt}")
            nc.tensor.matmul(ps2[:], ones[:], brow[0:1, HOC:],
                             start=True, stop=False)
            for k in range(3):
                kk = 128 if k < 2 else 64
                nc.tensor.matmul(ps2[:], sts[k][0:kk, :], w1p[0:kk, k, :],
                                 start=False, stop=(k == 2))
            h2 = sb.tile([128, D2C], f32, tag=f"h2{bt}", name=f"h2{bt}")
            nc.scalar.activation(h2[:], ps2[:], AF.Tanh)
            prod = sb.tile([128, D2C], f32, tag=f"prod{bt}", name=f"prod{bt}")
            nc.vector.tensor_tensor(prod[:], h2[:], w2b, op=ALU.mult)
            red = sb.tile([128, 5], f32, tag=f"red{bt}", name=f"red{bt}")
            nc.vector.tensor_reduce(
                red[:], prod[:].rearrange("p (h d) -> p h d", d=32),
                axis=mybir.AxisListType.X, op=ALU.add)
            nc.vector.tensor_tensor(lgs[:, bt, :], red[:], b2b, op=ALU.add)
        # single out DMA: src (p, bt, col) -> dram row bt*128+p
        dst = bass.AP(tensor=out_d.tensor, offset=0,
                      ap=[[5, 128], [128 * 5, 2], [1, 5]])
        nc.sync.dma_start(dst, lgs[:])

    nc.compile()
    _CACHE["nc"] = nc
    return nc


def _prep_inputs(x, coef, scale_base, scale_sp, lmd, W1, b1, W2, b2):
    polyc, tapS, tapR = _tables()
    xf = np.asarray(x, np.float32).reshape(B, I)

    coef = np.asarray(coef, np.float64)
    eff = coef * np.asarray(scale_sp, np.float64)[..., None] \
        * np.asarray(lmd, np.float64)[:, :, None, None]        # (H, I, O, 8)
    W = eff.transpose(1, 3, 0, 2).reshape(I, 8, H * O)         # (I, 8, 640)
    sbl = (np.asarray(scale_base, np.float64)
           * np.asarray(lmd, np.float64)[:, :, None]
           ).transpose(1, 0, 2).reshape(I, H * O)

    # silu(x) lies (to ~1e-6) in the span of the 8-fn spline basis: fit it
    # and fold sbl * beta into the plane weights -- no silu plane on device
    g = np.linspace(0.0, 1.0, 4097)[:-1]
    gc = g - 0.5
    phi = np.stack([np.ones_like(g), gc, gc**2, gc**3,
                    np.maximum(g - 0.6, 0)**3, np.maximum(g - 0.8, 0)**3,
                    np.maximum(0.2 - g, 0)**3, np.maximum(0.4 - g, 0)**3], 1)
    beta = np.linalg.lstsq(phi, g / (1 + np.exp(-g)), rcond=None)[0]

    # fold: device plane order xc, xc^2, xc^3, R3', R4', S1', S2'
    # (d = 5*xc, so d-basis folds scale by 5^s; cubes by 125)
    Wp = np.empty((I, NP, H * O))
    Wp[:, 0] = 5.0 * np.einsum('j,ijo->io', polyc[:, 1], W) + beta[1] * sbl
    Wp[:, 1] = 25.0 * np.einsum('j,ijo->io', polyc[:, 2], W) + beta[2] * sbl
    Wp[:, 2] = 125.0 * np.einsum('j,ijo->io', polyc[:, 3], W) + beta[3] * sbl
    Wp[:, 3] = 125.0 * np.einsum('j,ijo->io', tapR[:, 0], W) + beta[4] * sbl
    Wp[:, 4] = 125.0 * np.einsum('j,ijo->io', tapR[:, 1], W) + beta[5] * sbl
    Wp[:, 5] = 125.0 * np.einsum('j,ijo->io', tapS[:, 0], W) + beta[6] * sbl
    Wp[:, 6] = 125.0 * np.einsum('j,ijo->io', tapS[:, 1], W) + beta[7] * sbl
    bias_full = np.einsum('j,ijo->o', polyc[:, 0], W) \
        + beta[0] * sbl.sum(0)                                 # (640,)

    W1 = np.asarray(W1, np.float64)
    W2 = np.asarray(W2, np.float64).reshape(H * 32)
    b1 = np.asarray(b1, np.float64).reshape(H * 32)
    b2 = np.asarray(b2, np.float64).reshape(H)

    per_og = []
    for og in range(OG):
        hs = slice(og * HOC, (og + 1) * HOC)
        # weight stream: 8 plane pieces [128, 6*320] then packed chunk-6
        pieces = []
        for p in range(NP):
            blk = Wp[0:NCH * 128, p, hs].reshape(NCH, 128, HOC)
            pieces.append(np.ascontiguousarray(
                blk.transpose(1, 0, 2)).reshape(-1))
        w6 = np.zeros((NP * PL, HOC))
        for p in range(NP):
            w6[p * PL:(p + 1) * PL] = Wp[NCH * 128:I, p, hs]
        pieces.append(np.ascontiguousarray(w6).reshape(-1))
        wdev = np.concatenate(pieces).astype(np.float16)

        brow = np.zeros((1, HOC + D2C))
        brow[0, 0:HOC] = bias_full[hs]
        brow[0, HOC:] = b1[og * D2C:(og + 1) * D2C]
        brow = brow.astype(np.float16)

        w1bd = np.zeros((HOC, D2C))
        for hl in range(H // OG):
            w1bd[hl * O:(hl + 1) * O, hl * 32:(hl + 1) * 32] = W1[og * (H // OG) + hl]
        w1dev = np.zeros((128, 3, D2C))
        w1dev[:, 0] = w1bd[0:128]
        w1dev[:, 1] = w1bd[128:256]
        w1dev[0:64, 2] = w1bd[256:HOC]
        cf16 = np.concatenate([w1dev.reshape(128, 3 * D2C),
                               np.eye(128)], 1).astype(np.float16)
        cf32 = np.concatenate([
            np.broadcast_to(W2[og * D2C:(og + 1) * D2C], (128, D2C)),
            np.broadcast_to(b2[og * 5:(og + 1) * 5], (128, 5))],
            1).astype(np.float32)
        per_og.append((wdev, brow, cf16, cf32))

    in_maps = []
    for core in range(NC):
        bg, og = core % BG, core // BG
        xs = (xf[bg * BC:(bg + 1) * BC].T - 0.5).astype(np.float16)  # (784, 256)
        xdev = np.zeros((7, 128, BC), np.float16)
        xdev.reshape(7 * 128, BC)[0:I] = xs
        xdev = np.ascontiguousarray(xdev.transpose(1, 0, 2)).reshape(128, 7 * BC)
        wdev, brow, cf16, cf32 = per_og[og]
        in_maps.append({"x": xdev, "w": wdev, "brow": brow,
                        "cf16": cf16, "cf32": cf32})
    return in_maps


def run(inputs, trace=False, tmpdir=None):
    _install_ntff_hook()
    from concourse.bass_utils import run_bass_kernel_spmd
    nc = _build()
    in_maps = _prep_inputs(**inputs)
    res = run_bass_kernel_spmd(nc, in_maps, core_ids=list(range(NC)),
                               trace=trace, tmpdir=tmpdir)
    out = np.empty((B, H), np.float32)
    for core in range(NC):
        bg, og = core % BG, core // BG
        out[bg * BC:(bg + 1) * BC, og * 5:(og + 1) * 5] = res.results[core]["out"]
    return out, res


def kernel(**inputs):
    out, _ = run(inputs)
    return out
